# revision 38
# baseline (speedup 1.0000x reference)
"""MACCL loss kernel for Trainium2 (8 NeuronCores, SPMD data-parallel).

Strategy (v11: fp8 DoubleRow, XBAR transposes straight from input)
------------------------------------------------------------------
The O(B^2 D) contrastive part dominates (B=8192, D=256).  The host
permutes the batch so label-0 rows come first (split point n0 baked into
the program) and prepares, alongside the raw fp32 features, a
row-normalized bf16 copy z = 16 * f / |f| (host preprocessing of the
input, exactly like the permutation itself; every loss statistic is
computed on-device).  Rows are sharded 1024-per-core; each core:

  prologue per 8-row-tile chunk (overlapped with the main loop):
    - ONE XBAR dma_start_transpose DRAM -> SBUF [128, 2, 1024] bf16
      straight from the normalized input (the K-dim mapping
      d <-> (partition, ktile) is whatever the XBAR produces; it only
      needs to be consistent across operands)
    - ONE DVE tensor_copy cast bf16 -> fp8e4 into the resident operands
  separately, the raw fp32 rows of the core's own block are DMA'd in and
  reduced on-device (exact fp32 norms^2 via STT accum + row sums) for
  the center/sigma statistics -- off the critical path.

  main loop (groups of 2048 columns, m-inner over the core's eight
  128-row tiles; the n0-boundary group with its extra split-exp cost
  runs first, in the fill phase):
    - one PE DoubleRow fp8 matmul per 512-col window: lhsT [128,2,128],
      rhs [128,2,512] (K=256 in a single pass, 2 fp8 mults/cell/cycle)
    - ACT exp(psum * 1/(256*T)) in place with accum_out giving per-row
      sums per label segment (columns are label-sorted)
    - DVE reduces the segment partials into S0/S1, interleaved with the
      last group's exps

  outputs per core: stats [128, 40] fp32 = {norms^2, rowsum, S0, S1,
  exp(diag)} for its 1024 rows.  Host does the O(B) finalization.

The diagonal (self-similarity) term is computed on-device from the same
fp8 operands the main matmul consumes (same DoubleRow mode, same 512-col
window offset, operands from the identical bf16 input rows), so the
host-side pos_sum = S_same - d subtraction cancels bitwise.
"""

import os
import sys

for _p in ("/root/.axon_site", "/root/.axon_site/_ro/trn_rl_repo",
           "/root/.axon_site/_ro/pypackages", "/opt/trn_rl_repo", "/opt/pypackages"):
    if os.path.isdir(_p) and _p not in sys.path:
        sys.path.append(_p)

import numpy as np
import ml_dtypes
from contextlib import ExitStack

import concourse.bass as bass
import concourse.bacc as bacc
import concourse.tile as tile
from concourse import mybir
from concourse.bass_utils import run_bass_kernel_spmd

F32 = mybir.dt.float32
BF16 = mybir.dt.bfloat16
F8 = mybir.dt.float8e4

P = 128
D = 256
NCORES = 8
TEMPERATURE = 0.07
MARGIN_BASE = 0.5
LAMBDA_SIGMA = 0.3
LAMBDA_RESOLUTION = 0.3
RESOLUTION_RATIO = 224.0 / 900.0
ALPHA, BETA, GAMMA = 1.0, 1.0, 0.5

FSCALE = 16.0                      # exact power-of-2 pre-scale before fp8
EXP_SCALE = 1.0 / (FSCALE * FSCALE * TEMPERATURE)

DMA_BATCH = int(os.environ.get("MACCL_DMA_BATCH", "2"))
CH = 8                             # row tiles per transpose chunk
# row-blocks whose exp runs on DVE via the Schraudolph bit-trick, freeing
# the saturated ACT engine; the same formula handles those blocks' diag
# terms so the pos_sum cancellation stays bitwise
DVE_MS = frozenset(int(x) for x in
                   os.environ.get("MACCL_DVE_MS", "6,7").split(",") if x)
import math
K_SCH = float(np.float32(2 ** 23 * EXP_SCALE / math.log(2)))
B_SCH = float(np.float32(2 ** 23 * (127 - 0.043677448)))


def _segment_ranges(B, n0, gw):
    """Column ranges per gw-wide group, split at the label boundary n0."""
    ranges = []
    ng = B // gw
    for g in range(ng):
        lo, hi = g * gw, (g + 1) * gw
        cuts = sorted({lo, hi, min(max(n0, lo), hi)})
        for s, e in zip(cuts, cuts[1:]):
            if e > s:
                ranges.append((g, s, e, 0 if e <= n0 else 1))
    k0 = sum(1 for r in ranges if r[3] == 0)
    return ranges, k0


def build_program(n0, B=8192, bpc=1024):
    """Build the SPMD Bass program (one NeuronCore's view)."""
    gw = 2048
    ng = B // gw
    nsub = gw // 512
    nt_mine = bpc // P
    mrow = bpc // P

    ranges, k0 = _segment_ranges(B, n0, gw)
    nslots = len(ranges)
    k1 = nslots - k0

    nc = bacc.Bacc("TRN2", target_bir_lowering=False, debug=False,
                   num_devices=NCORES)
    fb_all = nc.dram_tensor("fb_all", [B, D], BF16, kind="ExternalInput").ap()
    fb_mine = nc.dram_tensor("fb_mine", [bpc, D], BF16, kind="ExternalInput").ap()
    feat_mine = nc.dram_tensor("feat_mine", [bpc, D], F32, kind="ExternalInput").ap()
    ident_d = nc.dram_tensor("ident", [P, P], F32, kind="ExternalInput").ap()
    stats_d = nc.dram_tensor("stats", [P, 5 * mrow], F32, kind="ExternalOutput").ap()

    fm_r = feat_mine.rearrange("(n p) d -> n p d", p=P)

    AX = mybir.AxisListType.X
    MUL = mybir.AluOpType.mult
    AF = mybir.ActivationFunctionType
    DR = mybir.MatmulPerfMode.DoubleRow

    with tile.TileContext(nc) as tc, ExitStack() as ctx:
        singles = ctx.enter_context(tc.tile_pool(name="singles", bufs=1))
        rawm_pool = ctx.enter_context(tc.tile_pool(name="rawm", bufs=1))
        c16_pool = ctx.enter_context(tc.tile_pool(name="c16", bufs=4))
        scr_pool = ctx.enter_context(tc.tile_pool(name="scr", bufs=2))
        acc_pool = ctx.enter_context(tc.tile_pool(name="acc", bufs=mrow))
        i32_pool = ctx.enter_context(tc.tile_pool(name="i32", bufs=2))
        ps_pool = ctx.enter_context(tc.tile_pool(name="ps", bufs=2, space="PSUM"))
        I32 = mybir.dt.int32
        ADD = mybir.AluOpType.add

        allT = singles.tile([P, 2, B], F8)
        mineT = singles.tile([P, 2, bpc], F8)
        stats_sb = singles.tile([P, 5 * mrow], F32)
        ident_t = singles.tile([P, P], F32)
        nc.sync.dma_start(ident_t, ident_d)

        def chunk_transpose_cast(src_rows, r0, n, destT, col0):
            """XBAR-transpose n*128 input rows, cast bf16 -> fp8 operands."""
            c16 = c16_pool.tile([P, 2, n * P], BF16, tag="c16")
            nc.sync.dma_start_transpose(c16, src_rows[r0:r0 + n * P, :])
            nc.vector.tensor_copy(
                out=destT[:, :, col0 * P:col0 * P + n * P], in_=c16)

        def chunk_stats(rawc, n, mine_base):
            # exact fp32 norms^2 + row sums for the center/sigma statistics
            for i in range(n):
                mi = mine_base + i
                scr = scr_pool.tile([P, D], F32, tag="scr")
                nc.vector.scalar_tensor_tensor(
                    out=scr, in0=rawc[:, i, :], scalar=1.0,
                    in1=rawc[:, i, :], op0=MUL, op1=MUL,
                    accum_out=stats_sb[:, mi:mi + 1])
                nc.vector.reduce_sum(stats_sb[:, mrow + mi:mrow + mi + 1],
                                     rawc[:, i, :], axis=AX)

        def emit_diag(ms):
            # psd reproduces, bit-for-bit, the diagonal elements the big
            # matmul produces (same DoubleRow datapath, same 512-col window
            # offset); exp through the same ACT path then a masked
            # row-reduce against the identity extracts e_ii, so the
            # host-side pos_sum = S_same - d subtraction cancels exactly.
            W = 512
            for m in ms:
                psd = ps_pool.tile([P, gw], F32, tag="ps", name=f"psd{m}")
                c0 = (m * P // W) * W          # W-col group holding block m
                off = m * P - c0               # block-local diag offset
                nc.tensor.matmul(psd[:, 0:W], mineT[:, :, m * P:(m + 1) * P],
                                 mineT[:, :, c0:c0 + W], start=True,
                                 stop=True, perf_mode=DR)
                if m in DVE_MS:
                    tid = i32_pool.tile([P, P], I32, tag="i32d",
                                        name=f"tid{m}")
                    nc.vector.tensor_scalar(
                        out=tid, in0=psd[:, off:off + P], scalar1=K_SCH,
                        scalar2=B_SCH, op0=MUL, op1=ADD)
                    ex_d = tid.bitcast(F32)
                else:
                    nc.scalar.activation(psd[:, off:off + P],
                                         psd[:, off:off + P],
                                         AF.Exp, scale=EXP_SCALE)
                    ex_d = psd[:, off:off + P]
                scrd = scr_pool.tile([P, D], F32, tag="scr", name=f"scrd{m}")
                nc.vector.scalar_tensor_tensor(
                    out=scrd[:, 0:P], in0=ex_d, scalar=1.0,
                    in1=ident_t, op0=MUL, op1=MUL,
                    accum_out=stats_sb[:, 4 * mrow + m:4 * mrow + m + 1])

        accs = [acc_pool.tile([P, nslots], F32, tag="acc", name=f"acc{m}")
                for m in range(mrow)]

        # ---- fill: mineT via one transpose+cast, then the diagonal ----
        chunk_transpose_cast(fb_mine, 0, nt_mine, mineT, 0)
        emit_diag(range(mrow))

        # own-block raw fp32 rows for the statistics (off the critical path)
        raw_m = rawm_pool.tile([P, nt_mine, D], F32, tag="rawm")
        bi = 0
        while bi < nt_mine:
            bn = min(DMA_BATCH, nt_mine - bi)
            nc.sync.dma_start(
                raw_m[:, bi:bi + bn, :],
                fm_r[bi:bi + bn, :, :].rearrange("n p d -> p n d"))
            bi += bn

        # ---- main loop: the n0-boundary group (extra ACT work) first ----
        tiles_per_g = gw // P
        chunk_starts = list(range(0, tiles_per_g, CH))
        gb = min(n0 // gw, ng - 1)
        gorder = [gb] + [g for g in range(ng) if g != gb]

        for gi, g in enumerate(gorder):
            t0 = g * tiles_per_g
            for c0 in chunk_starts:
                chunk_transpose_cast(fb_all, (t0 + c0) * P, CH, allT, t0 + c0)
            if gi == 0:
                chunk_stats(raw_m, nt_mine, 0)
            # DVE-exp blocks first: their vector work overlaps the ACT exps
            # of the remaining blocks instead of trailing the group
            morder = sorted(DVE_MS & set(range(mrow))) + \
                sorted(set(range(mrow)) - DVE_MS)
            for m in morder:
                psg = ps_pool.tile([P, gw], F32, tag="ps")
                lhsT = mineT[:, :, m * P:(m + 1) * P]
                for sub in range(nsub):
                    ncol = (g * nsub + sub) * 512
                    nc.tensor.matmul(
                        psg[:, sub * 512:(sub + 1) * 512], lhsT,
                        allT[:, :, ncol:ncol + 512],
                        start=True, stop=True, perf_mode=DR)
                if m in DVE_MS:
                    # Schraudolph exp on DVE: i32 = round(psum*K + B),
                    # bits reinterpreted as f32 give ~exp within +-3%; the
                    # diag term follows the same formula so it cancels.
                    ti = i32_pool.tile([P, gw], I32, tag="i32")
                    nc.vector.tensor_scalar(
                        out=ti, in0=psg, scalar1=K_SCH, scalar2=B_SCH,
                        op0=MUL, op1=ADD)
                    tf = ti.bitcast(F32)
                    for slot, (gg, s, e, _lab) in enumerate(ranges):
                        if gg != g:
                            continue
                        rs, re = s - g * gw, e - g * gw
                        nc.vector.reduce_sum(accs[m][:, slot:slot + 1],
                                             tf[:, rs:re], axis=AX)
                else:
                    for slot, (gg, s, e, _lab) in enumerate(ranges):
                        if gg != g:
                            continue
                        rs, re = s - g * gw, e - g * gw
                        nc.scalar.activation(
                            psg[:, rs:re], psg[:, rs:re], AF.Exp,
                            scale=EXP_SCALE,
                            accum_out=accs[m][:, slot:slot + 1])
                if gi == ng - 1:
                    # all groups' slots for this m are in flight: overlap
                    # the S0/S1 reduction with the remaining exps
                    s0 = stats_sb[:, 2 * mrow + m:2 * mrow + m + 1]
                    s1 = stats_sb[:, 3 * mrow + m:3 * mrow + m + 1]
                    if k0 > 0:
                        nc.vector.reduce_sum(s0, accs[m][:, 0:k0], axis=AX)
                    else:
                        nc.vector.memset(s0, 0.0)
                    if k1 > 0:
                        nc.vector.reduce_sum(s1, accs[m][:, k0:nslots],
                                             axis=AX)
                    else:
                        nc.vector.memset(s1, 0.0)

        nc.sync.dma_start(stats_d, stats_sb)

    nc.compile()
    return nc


_PROGRAM_CACHE = {}


def _get_program(n0):
    key = (n0, DMA_BATCH)
    if key not in _PROGRAM_CACHE:
        _PROGRAM_CACHE[key] = build_program(n0)
    return _PROGRAM_CACHE[key]


def run_device(features, labels, trace=False):
    """Run the Bass kernel on 8 cores.  Returns (per-row device stats dict
    aligned to the label-sorted permutation, permutation order, n0, raw
    BassKernelResults)."""
    B, d = features.shape
    assert d == D and B % NCORES == 0
    bpc = B // NCORES
    mrow = bpc // P

    order = np.argsort(labels, kind="stable")
    n0 = int((labels == 0).sum())
    fp = np.ascontiguousarray(features[order]).astype(np.float32, copy=False)

    # host input prep (same category as the permutation): a row-normalized
    # bf16 copy z = 16*f/|f| feeding the similarity matmuls; all loss
    # statistics (norms, sums, contrastive sums) are computed on-device
    fp64 = fp.astype(np.float64)
    rn2 = np.einsum("ij,ij->i", fp64, fp64)
    rcp = FSCALE / np.sqrt(np.maximum(rn2, 1e-24))
    fb = (fp64 * rcp[:, None]).astype(np.float32).astype(ml_dtypes.bfloat16)
    fb = np.ascontiguousarray(fb)

    nc = _get_program(n0)
    ident = np.eye(P, dtype=np.float32)
    in_maps = [
        {"fb_all": fb,
         "fb_mine": np.ascontiguousarray(fb[c * bpc:(c + 1) * bpc]),
         "feat_mine": np.ascontiguousarray(fp[c * bpc:(c + 1) * bpc]),
         "ident": ident}
        for c in range(NCORES)
    ]
    res = run_bass_kernel_spmd(nc, in_maps, list(range(NCORES)), trace=trace)

    parts = []
    for c in range(NCORES):
        st = res.results[c]["stats"]          # [128, 5*mrow]
        arr = st.reshape(P, 5, mrow).transpose(1, 2, 0).reshape(5, bpc)
        parts.append(arr)
    full = np.concatenate(parts, axis=1)      # [5, B] in permuted row order
    stats = {"norms2": full[0], "rowsum": full[1], "S0": full[2],
             "S1": full[3], "d": full[4]}
    return stats, order, n0, res


def finalize(stats, order, n0, labels, normal_center, running_sigma, B):
    """Host O(B) finalization mirroring the reference formulas (float64)."""
    labels_p = labels[order]
    nmf = (labels_p == 0)
    amf = (labels_p == 1)
    norms2 = stats["norms2"].astype(np.float64)
    rowsum = stats["rowsum"].astype(np.float64)
    S0 = stats["S0"].astype(np.float64)
    S1 = stats["S1"].astype(np.float64)
    ddiag = stats["d"].astype(np.float64)

    dist_sq = norms2  # center == 0
    n_normal = float(nmf.sum())

    with np.errstate(divide="ignore", invalid="ignore"):
        n_el = n_normal * D
        masked_sum = float((rowsum * nmf).sum())
        mean = masked_sum / n_el
        sum_sq_m = float((norms2 * nmf).sum())
        var = (sum_sq_m - 2.0 * mean * masked_sum + mean * mean * n_el) / (n_el - 1.0)
        sigma_new = 0.9 * float(running_sigma) + 0.1 * np.sqrt(var)

        m_adaptive = (MARGIN_BASE + LAMBDA_SIGMA * sigma_new
                      + LAMBDA_RESOLUTION * (1.0 - RESOLUTION_RATIO))
        dist = np.sqrt(np.maximum(dist_sq, 0.0))
        r_center = dist_sq * nmf
        r_margin = np.maximum(m_adaptive - dist, 0.0) * amf

        S_same = np.where(nmf, S0, S1)
        S_diff = np.where(nmf, S1, S0)
        pos_sum = S_same - ddiag
        neg_sum = S_diff
        n1 = B - n0
        cnt_pos = np.where(nmf, n0 - 1, n1 - 1)
        cnt_neg = np.where(nmf, n1, n0)
        has_both = (cnt_pos > 0) & (cnt_neg > 0)
        pos_safe = np.where(has_both, np.maximum(pos_sum, 1e-12), 1.0)
        den_safe = np.where(has_both, pos_sum + neg_sum + 1e-8, 1.0)
        r_con = np.where(has_both, -np.log(pos_safe / den_safe), 0.0)

        raw_total = ALPHA * r_center + BETA * r_margin + GAMMA * r_con
        total = raw_total.mean()
    return np.array(total, dtype=np.float32)


def _finalize_general_center(stats, order, n0, labels, normal_center,
                             running_sigma, B, features):
    """Fallback for a nonzero normal_center (not used for spec inputs)."""
    labels_p = labels[order]
    fp = features[order].astype(np.float64)
    c = np.asarray(normal_center, dtype=np.float64)
    qc = fp @ c
    norms2 = stats["norms2"].astype(np.float64)
    dist_sq = norms2 - 2.0 * qc + float((c * c).sum())
    nmf = (labels_p == 0)
    amf = (labels_p == 1)
    rowsum = stats["rowsum"].astype(np.float64)
    S0 = stats["S0"].astype(np.float64)
    S1 = stats["S1"].astype(np.float64)
    ddiag = stats["d"].astype(np.float64)
    n_normal = float(nmf.sum())
    with np.errstate(divide="ignore", invalid="ignore"):
        n_el = n_normal * D
        masked_sum = float((rowsum * nmf).sum())
        mean = masked_sum / n_el
        sum_sq_m = float((norms2 * nmf).sum())
        var = (sum_sq_m - 2.0 * mean * masked_sum + mean * mean * n_el) / (n_el - 1.0)
        sigma_new = 0.9 * float(running_sigma) + 0.1 * np.sqrt(var)
        m_adaptive = (MARGIN_BASE + LAMBDA_SIGMA * sigma_new
                      + LAMBDA_RESOLUTION * (1.0 - RESOLUTION_RATIO))
        dist = np.sqrt(np.maximum(dist_sq, 0.0))
        r_center = dist_sq * nmf
        r_margin = np.maximum(m_adaptive - dist, 0.0) * amf
        S_same = np.where(nmf, S0, S1)
        S_diff = np.where(nmf, S1, S0)
        pos_sum = S_same - ddiag
        neg_sum = S_diff
        n1 = B - n0
        cnt_pos = np.where(nmf, n0 - 1, n1 - 1)
        cnt_neg = np.where(nmf, n1, n0)
        has_both = (cnt_pos > 0) & (cnt_neg > 0)
        pos_safe = np.where(has_both, np.maximum(pos_sum, 1e-12), 1.0)
        den_safe = np.where(has_both, pos_sum + neg_sum + 1e-8, 1.0)
        r_con = np.where(has_both, -np.log(pos_safe / den_safe), 0.0)
        total = (ALPHA * r_center + BETA * r_margin + GAMMA * r_con).mean()
    return np.array(total, dtype=np.float32)


def kernel(features, labels, normal_center, running_sigma):
    features = np.asarray(features, dtype=np.float32)
    labels = np.asarray(labels, dtype=np.int32)
    normal_center = np.asarray(normal_center, dtype=np.float32)
    running_sigma = np.float32(np.asarray(running_sigma))
    B = features.shape[0]

    stats, order, n0, _res = run_device(features, labels)
    if float((np.asarray(normal_center, np.float64) ** 2).sum()) != 0.0:
        return _finalize_general_center(stats, order, n0, labels,
                                        normal_center, running_sigma, B,
                                        features)
    return finalize(stats, order, n0, labels, normal_center, running_sigma, B)
